# revision 54
# baseline (speedup 1.0000x reference)
"""Trainium2 Bass kernel for nn_EnergyDistributionCNN (3x3 conv -> unfold ->
softmax over patch -> weighted -> fold overlap-add), 8 NeuronCores.

Math (algebraically identical to the torch/jax reference):
    out = conv3x3(x, k)            cross-correlation, zero pad 1
    E   = exp(out)                 (pad pixels contribute exp(0)=1)
    Z   = boxsum3x3(E with ones at pad)
    U   = x / Z
    S   = boxsum3x3(U zero-padded)
    result = E * S

Sharding: row-block across 8 cores with a 3-row halo sliced on the host
(zero-filled at the global edges) -- no device-to-device communication.

Pipeline is fp16 end-to-end: the host casts x to fp16 (rel err 2.4e-4,
well inside the 2e-2 tolerance; measured end-to-end pipeline error is
~3e-3), which halves HBM traffic, runs the PE at full rate and unlocks
the DVE 2x packed-16-bit mode.  Work is spread over all five engines:

  PE   : conv (3 band-matmul passes), Z box (3 shifted ones-band passes,
         or 1 pass on 'mix' tiles), S vertical (1 pass)
  Act  : exp from conv's PSUM (with the row-validity mask folded into the
         scale operand: exp(0*junk)=1), and the S PSUM->fp16 copy
  DVE  : U = x/Z as a single tensor_tensor divide straight from PSUM,
         second horizontal add of the U box, final res = E*S fp16 mul,
         (on 'mix' tiles also the horizontal E adds)
  Pool : first horizontal add of the U box (GpSimd is otherwise idle)
  DMA  : fp16 loads/stores

Row mapping keeps every compute op at partition base 0 (hardware only
allows compute APs to start at partitions 0/32/64/96): the banded
matrices alternate lower/upper diagonals so each stage's output lands
re-centred, and frame-edge partitions hold junk that is either masked
(exp scale), harmless (finite, unused), or skipped by the output DMA
(which may start at any partition).
"""

from contextlib import ExitStack

import numpy as np

import concourse.bacc as bacc
import concourse.mybir as mybir
import concourse.tile as tile
from concourse._compat import with_exitstack
from concourse.bass_utils import run_bass_kernel_spmd
from concourse import dve_ops as _dve_ops
from concourse.dve_spec import AluOp as _AluOp
from concourse.dve_spec import Bin as _Bin
from concourse.dve_spec import C0 as _C0
from concourse.dve_spec import C1 as _C1
from concourse.dve_spec import Spec as _Spec
from concourse.dve_spec import Src0 as _Src0
from concourse.dve_spec import Src1 as _Src1
from concourse.dve_spec import lower as _dve_lower
from concourse.dve_uop import DveOpSpec as _DveOpSpec


def _register_recip1nr_mul():
    """Fused out = in1 * approx(1/in0) as ONE custom-DVE op: the fp32
    exponent-flip seed (x*bitcast(~x) lands in [-4.5,-4]), one Chebyshev
    scale and a single Newton pass (~1.7e-3 rel err, fine for the 2e-2
    tolerance), then the multiply -- 6 ALU stages, fits the 8-stage pipe.
    Replaces reciprocal_approx_fast + tensor_mul (halves the DVE cost of
    the U = x/Z stage)."""
    name = "RECIP1NR_MUL_ANT"
    for op in _dve_ops.OPS:
        if op.name == name:
            return op
    _not = _Bin(_AluOp.BITWISE_NOT, _Src0, _Src0)
    _y0 = _not * _C0
    _y1 = _y0 * (_C1 - _Src0 * _y0)

    def _ref(in0, in1, c0, c1, c2):
        not_x = (~np.asarray(in0, np.float32).view(np.int32)).view(np.float32)
        y0 = not_x * np.float32(c0)
        y1 = y0 * (np.float32(c1) - in0 * y0)
        return y1 * in1

    spec = _Spec(body=_y1 * _Src1, reference=_ref)
    row = max(_dve_ops._SUB_OPCODE_FOR_NAME.values()) + 1
    assert row < 0x20
    _dve_ops._SUB_OPCODE_FOR_NAME[name] = row
    shas = {}
    for ver in ("v3", "v4"):
        try:
            uops = _dve_lower(spec, ver=ver)
        except Exception:
            continue
        shas[ver] = _DveOpSpec(name=name, opcode=row, uops=uops, rd1_en=True).sha(ver)
    op = _dve_ops.DveOp(name, spec, subdim=False, uops_sha=shas)
    _dve_ops.OPS.append(op)
    _dve_ops.CUSTOM_DVE_SPECS[name] = spec
    return op


RECIP1NR_MUL = _register_recip1nr_mul()
RECIP_C = _dve_ops.RECIP_APPROX_FAST_CONSTS

F16 = mybir.dt.float16
F32 = mybir.dt.float32

H = 4096
W = 4096
N_CORES = 8
RC = H // N_CORES  # rows per core
HALO = 3
RT = 122           # output rows per normal row-tile (RT + 6 <= 128)
C = 512            # matmul column chunk = one fp32 PSUM bank
CG = 1024          # conv PSUM group (2 banks) -> fewer Act instructions
SEGW = 1024        # folded-tile width segment (4 segs on 32-row blocks)

# Which normal tiles compute the Z box via horizontal adds + 1 matmul
# instead of 3 shifted matmuls on the PE (engine balancing knob).
ZMIX = (False, True, False, False)

XW = W + 2 * HALO        # X cols:  j   <-> global col j-3   (4102)
EW = W + 4               # E cols:  e   <-> global col e-2   (4100)
ZW = W + 2               # Z/U cols: c  <-> global col c-1   (4098)


# ---------------------------------------------------------------- host side

def _band(vals, lo):
    """128x128 band matrix: b[p, m] = vals[p-m-lo] for p-m-lo in 0..2."""
    b = np.zeros((128, 128), np.float32)
    idx = np.arange(128)
    for d in range(3):
        off = lo + d
        p = idx[off:] if off >= 0 else idx[: 128 + off]
        m = p - off
        b[p, m] = vals[d]
    return b


def _make_bands(k: np.ndarray) -> np.ndarray:
    """bands[0..2]: conv lhsT per column-shift v (b[p,m]=k[p-m, v]);
    bands[3]: BT ones, lhsT[m,p]=1 for p-m in 0..2 (Z: E-frame -> X-frame);
    bands[4]: BS ones, lhsT[p,m]=1 for p-m in 0..2 (S: X-frame -> E-frame);
    bands[5..9]: the same five as 4x 32x32 block-diagonals (folded tile)."""
    bands = np.zeros((10, 128, 128), np.float32)
    for v in range(3):
        bands[v] = _band(k[:, v], 0)
    bands[3] = _band([1.0, 1.0, 1.0], 0).T
    bands[4] = _band([1.0, 1.0, 1.0], 0)
    for i in range(5):
        for b in range(4):
            s = slice(32 * b, 32 * b + 32)
            bands[5 + i][s, s] = bands[i][:32, :32]
    # p-major [128, 10*128] so the upload is one contiguous 2560B/partition DMA
    return np.ascontiguousarray(bands.transpose(1, 0, 2).reshape(128, -1)).astype(
        np.float16
    )


def _make_core_inputs(x16: np.ndarray, bands: np.ndarray, core: int):
    r0 = core * RC
    lo, hi = r0 - HALO, r0 + RC + HALO
    xh = np.zeros((RC + 2 * HALO, XW), np.float16)
    s_lo, s_hi = max(lo, 0), min(hi, H)
    xh[s_lo - lo : s_hi - lo, HALO : HALO + W] = x16[s_lo:s_hi]
    gl = np.arange(lo, hi)
    mask = ((gl >= 0) & (gl < H)).astype(np.float32)[:, None]
    # folded-tile inputs pre-blocked on the host: 32-row x (SEGW+6)-col
    # window per 32-partition block -> single contiguous DMAs on device
    fo = RC - 24
    xf = np.zeros((128, SEGW + 6), np.float16)
    mf = np.zeros((128, 1), np.float32)
    for b in range(4):
        rows = xh[fo : fo + 32, b * SEGW : b * SEGW + SEGW + 6]
        xf[32 * b : 32 * b + rows.shape[0], :] = rows
        mf[32 * b : 32 * b + 28, 0] = mask[fo + 1 : fo + 29, 0]
    return {"xh": xh, "mask": mask, "bands": bands, "xf": xf, "mkf": mf}


def _chunks(total: int, step: int):
    out = []
    s = 0
    while s < total:
        out.append((s, min(step, total - s)))
        s += step
    return out


# -------------------------------------------------------------- device side

@with_exitstack
def _energy_body(ctx: ExitStack, tc, out_d, xh_d, mask_d, bands_d, xf_d, mkf_d):
    nc = tc.nc
    Exp = mybir.ActivationFunctionType.Exp
    Copy = mybir.ActivationFunctionType.Copy

    def u_div(out_ap, z_ap, x_ap):
        # U = x * approx(1/Z), one fused custom-DVE instruction straight
        # from Z's PSUM (fp16 output: the DVE output stage downconverts,
        # the fp32 bit-trick only concerns in0)
        nc.vector._custom_dve(
            RECIP1NR_MUL, out=out_ap, in0=z_ap, in1=x_ap,
            s0=RECIP_C["s0"], s1=RECIP_C["s1"], imm2=0.0,
        )

    consts = ctx.enter_context(tc.tile_pool(name="consts", bufs=1))
    bigb = consts.tile([128, 10 * 128], F16, name="bigb")
    nc.sync.dma_start(out=bigb, in_=bands_d)
    # warm the PE p-state on zeroed scratch while the first DMAs land
    wl = consts.tile([128, 128], F16, name="wl")
    wr = consts.tile([128, C], F16, name="wr")
    nc.vector.memset(wl, 0.0)
    nc.vector.memset(wr, 0.0)
    Mv = [bigb[:, i * 128 : (i + 1) * 128] for i in range(3)]
    BT = bigb[:, 3 * 128 : 4 * 128]
    BS = bigb[:, 4 * 128 : 5 * 128]
    MvF = [bigb[:, (5 + i) * 128 : (6 + i) * 128] for i in range(3)]
    BTF = bigb[:, 8 * 128 : 9 * 128]
    BSF = bigb[:, 9 * 128 : 10 * 128]

    xpool = ctx.enter_context(tc.tile_pool(name="xp", bufs=3))
    epool = ctx.enter_context(tc.tile_pool(name="ep", bufs=1))
    ehpool = ctx.enter_context(tc.tile_pool(name="ehp", bufs=2))
    upool = ctx.enter_context(tc.tile_pool(name="up", bufs=2))
    uhpool = ctx.enter_context(tc.tile_pool(name="uhp", bufs=2))
    spool = ctx.enter_context(tc.tile_pool(name="sp", bufs=2))
    respool = ctx.enter_context(tc.tile_pool(name="resp", bufs=3))
    mpool = ctx.enter_context(tc.tile_pool(name="mp", bufs=2))
    ps_c = ctx.enter_context(tc.tile_pool(name="psc", bufs=2, space="PSUM"))
    ps_z = ctx.enter_context(tc.tile_pool(name="psz", bufs=2, space="PSUM"))
    ps_s = ctx.enter_context(tc.tile_pool(name="pss", bufs=2, space="PSUM"))

    E_tiles = [epool.tile([128, EW], F16, name=f"E{i}") for i in range(3)]

    def normal_early(o, R, ti, zmix):
        """DMA, conv+exp, Z box, reciprocal.  Returns state for the late
        stage."""
        P = R + 4  # working partitions (E frame); X uses R+6
        mk = mpool.tile([128, 1], F32, tag="mk")
        nc.sync.dma_start(out=mk[:P], in_=mask_d[o + 1 : o + 1 + P, :])

        X = xpool.tile([128, XW], F16, tag="X")
        # quarters, so the first conv groups start after 1/4 of the transfer
        xq = [0, 2 + W // 4, 2 + W // 2, 2 + 3 * W // 4, XW]
        for a, b in zip(xq[:-1], xq[1:]):
            nc.sync.dma_start(out=X[: R + 6, a:b], in_=xh_d[o : o + R + 6, a:b])

        # conv + exp -> E[m, e] <-> (row o-2+m, col e-2); only the interior
        # e in [2, EW-2) is computed, the pad columns stay at their
        # pre-initialized exp(0)=1
        E = E_tiles[ti % 3]
        for g0, gl in _chunks(W, CG):
            pc = ps_c.tile([128, CG], F32, tag="pc")
            for cs, cl in _chunks(gl, C):
                for v in range(3):
                    nc.tensor.matmul(
                        pc[:P, cs : cs + cl],
                        Mv[v][: R + 6, :P],
                        X[: R + 6, g0 + 2 + cs + v : g0 + 2 + cs + v + cl],
                        start=(v == 0),
                        stop=(v == 2),
                    )
            nc.scalar.activation(
                E[:P, g0 + 2 : g0 + 2 + gl], pc[:P, :gl], Exp, scale=mk[:P]
            )

        if zmix:
            # horizontal E box on DVE in halves, vertical on PE (1 pass)
            eh1 = ehpool.tile([128, ZW], F16, tag="eh1")
            eh = ehpool.tile([128, ZW], F16, tag="eh")
            for h0, hl in ((0, ZW // 2), (ZW // 2, ZW - ZW // 2)):
                nc.vector.tensor_add(
                    out=eh1[:P, h0 : h0 + hl],
                    in0=E[:P, h0 : h0 + hl],
                    in1=E[:P, h0 + 1 : h0 + 1 + hl],
                )
                nc.vector.tensor_add(
                    out=eh[:P, h0 : h0 + hl],
                    in0=eh1[:P, h0 : h0 + hl],
                    in1=E[:P, h0 + 2 : h0 + 2 + hl],
                )

        # Z (X frame via BT), U = x/Z fused straight from PSUM
        U = upool.tile([128, ZW], F16, tag="U")
        for cs, cl in _chunks(ZW, C):
            pz = ps_z.tile([128, C], F32, tag="pz")
            if zmix:
                nc.tensor.matmul(
                    pz[:P, :cl], BT[:P, :P], eh[:P, cs : cs + cl],
                    start=True, stop=True,
                )
            else:
                for v in range(3):
                    nc.tensor.matmul(
                        pz[:P, :cl], BT[:P, :P], E[:P, cs + v : cs + v + cl],
                        start=(v == 0), stop=(v == 2),
                    )
            u_div(U[:P, cs : cs + cl], pz[:P, :cl], X[:P, cs + 2 : cs + 2 + cl])
        return o, R, P, X, E, U

    def normal_late(state, nq=4):
        """Horizontal U box (first add on the otherwise-idle GpSimd), S
        vertical (E frame via BS), PSUM -> fp16 via Act copy, res = E*S.
        Split into column quarters so the chain pipelines at ~1us
        granularity and the PE never starves long enough to drop out of
        its warm p-state."""
        o, R, P, X, E, U = state
        uh1 = uhpool.tile([128, W], F16, tag="uh1")
        uh = uhpool.tile([128, W], F16, tag="uh")
        S16 = spool.tile([128, W], F16, tag="S16")
        res = respool.tile([128, W], F16, tag="res")
        QW = W // nq
        for qi in range(nq):
            h0 = qi * QW
            nc.gpsimd.tensor_add(
                out=uh1[:P, h0 : h0 + QW],
                in0=U[:P, h0 : h0 + QW],
                in1=U[:P, h0 + 1 : h0 + 1 + QW],
            )
            nc.vector.tensor_add(
                out=uh[:P, h0 : h0 + QW],
                in0=uh1[:P, h0 : h0 + QW],
                in1=U[:P, h0 + 2 : h0 + 2 + QW],
            )
            for cs, cl in _chunks(QW, C):
                ps = ps_s.tile([128, C], F32, tag="ps")
                nc.tensor.matmul(
                    ps[: R + 2, :cl], BS[:P, : R + 2],
                    uh[:P, h0 + cs : h0 + cs + cl],
                    start=True, stop=True,
                )
                nc.scalar.activation(
                    S16[: R + 2, h0 + cs : h0 + cs + cl], ps[: R + 2, :cl], Copy
                )
            nc.vector.tensor_mul(
                out=res[: R + 2, h0 : h0 + QW],
                in0=E[: R + 2, h0 + 2 : h0 + 2 + QW],
                in1=S16[: R + 2, h0 : h0 + QW],
            )
            if qi % 2 == 1 or nq == 8:
                # valid output rows sit at partitions [2, R+2); one DMA per
                # half keeps the serialized HWDGE issue cost down
                w0 = h0 if nq == 8 else h0 - QW
                # issue from the Act queue: its wait (on resmul, just done)
                # is short there, and long input-load waits on the SP queue
                # can no longer block result stores (res-slot backpressure)
                nc.sync.dma_start(
                    out=out_d[o : o + R, w0 : h0 + QW],
                    in_=res[2 : R + 2, w0 : h0 + QW],
                )

    def fold_unit(o, R, p0):
        """The folded last row-tile: four 1024-wide segments stacked on
        32-partition blocks, block-diagonal bands, inputs pre-blocked on
        the host (single DMAs).  Emitted twice -- at both pipeline edges
        for fast fill/drain -- with each emission storing only the two
        segments at [p0, p0+64).  The compute is cheap enough (~1us per
        engine) that duplicating it beats quadrant-sliced matmuls."""
        mk = mpool.tile([128, 1], F32, tag="mkf")
        nc.sync.dma_start(out=mk, in_=mkf_d)
        X = xpool.tile([128, SEGW + 6], F16, tag="Xf")
        nc.sync.dma_start(out=X, in_=xf_d)

        ew, zw = SEGW + 4, SEGW + 2
        E = epool.tile([128, ew], F16, tag="Ef")
        for g0, gl in _chunks(ew, CG):
            pc = ps_c.tile([128, CG], F32, tag="pc")
            for cs, cl in _chunks(gl, C):
                for v in range(3):
                    nc.tensor.matmul(
                        pc[:, cs : cs + cl],
                        MvF[v],
                        X[:, g0 + cs + v : g0 + cs + v + cl],
                        start=(v == 0),
                        stop=(v == 2),
                    )
            nc.scalar.activation(E[:, g0 : g0 + gl], pc[:, :gl], Exp, scale=mk)
        nc.vector.memset(E[0:32, 1:2], 1.0)
        nc.vector.memset(E[96:128, ew - 2 : ew - 1], 1.0)

        U = upool.tile([128, zw], F16, tag="Uf")
        for cs, cl in _chunks(zw, C):
            pz = ps_z.tile([128, C], F32, tag="pz")
            for v in range(3):
                nc.tensor.matmul(
                    pz[:, :cl], BTF, E[:, cs + v : cs + v + cl],
                    start=(v == 0), stop=(v == 2),
                )
            u_div(U[:, cs : cs + cl], pz[:, :cl], X[:, cs + 2 : cs + 2 + cl])
        uh1 = uhpool.tile([128, SEGW], F16, tag="uh1f")
        nc.gpsimd.tensor_add(out=uh1, in0=U[:, 0:SEGW], in1=U[:, 1 : SEGW + 1])
        uh = uhpool.tile([128, SEGW], F16, tag="uhf")
        nc.vector.tensor_add(out=uh, in0=uh1, in1=U[:, 2 : SEGW + 2])

        S16 = spool.tile([128, SEGW], F16, tag="S16f")
        for cs, cl in _chunks(SEGW, C):
            ps = ps_s.tile([128, C], F32, tag="ps")
            nc.tensor.matmul(ps[:, :cl], BSF, uh[:, cs : cs + cl], start=True, stop=True)
            nc.scalar.activation(S16[:, cs : cs + cl], ps[:, :cl], Copy)
        res = respool.tile([128, SEGW], F16, tag="resf")
        nc.vector.tensor_mul(out=res, in0=E[:, 2 : SEGW + 2], in1=S16)
        return res

    def fold_store(res, o, R, blocks):
        for b in blocks:
            nc.sync.dma_start(
                out=out_d[o : o + R, b * SEGW : (b + 1) * SEGW],
                in_=res[32 * b + 2 : 32 * b + 2 + R, :],
            )

    with nc.allow_low_precision("fp16 pipeline; verified within tolerance"):
        # pre-initialized E tiles, rotated manually: the pad columns
        # (exp(0)=1 outside the grid) are set once and exp only ever writes
        # the interior, so no per-tile DVE memset sits in front of the next
        # tile's Z matmuls
        for Et in E_tiles:
            nc.vector.memset(Et[:, 0:2], 1.0)
            nc.vector.memset(Et[:, EW - 2 : EW], 1.0)
        for i in range(6):
            pw = ps_z.tile([128, C], F32, tag="pz")
            nc.tensor.matmul(pw, wl, wr, start=True, stop=True)

        tiles = _chunks(RC, RT)
        fo, fr = tiles[-1]
        assert fr <= 26
        # folded tile first (cheap fill); half its stores drain at the end
        resf = fold_unit(fo, fr, 0)
        fold_store(resf, fo, fr, (0, 1))
        nbig = len(tiles) - 1
        for i, (o, R) in enumerate(tiles[:-1]):
            st = normal_early(o, R, i, ZMIX[i % len(ZMIX)])
            normal_late(st, nq=8 if i == nbig - 1 else 4)
        fold_store(resf, fo, fr, (2, 3))


_CACHE: dict = {}


def _build():
    if "nc" in _CACHE:
        return _CACHE["nc"]
    nc = bacc.Bacc(
        "TRN2", target_bir_lowering=False, debug=False, num_devices=N_CORES
    )
    xh_d = nc.dram_tensor("xh", (RC + 2 * HALO, XW), F16, kind="ExternalInput").ap()
    mask_d = nc.dram_tensor("mask", (RC + 2 * HALO, 1), F32, kind="ExternalInput").ap()
    bands_d = nc.dram_tensor("bands", (128, 10 * 128), F16, kind="ExternalInput").ap()
    xf_d = nc.dram_tensor("xf", (128, SEGW + 6), F16, kind="ExternalInput").ap()
    mkf_d = nc.dram_tensor("mkf", (128, 1), F32, kind="ExternalInput").ap()
    out_d = nc.dram_tensor("out", (RC, W), F16, kind="ExternalOutput").ap()
    with tile.TileContext(nc) as tc:
        _energy_body(tc, out_d, xh_d, mask_d, bands_d, xf_d, mkf_d)
    nc.compile()
    _CACHE["nc"] = nc
    return nc


def kernel(shareable_energy: np.ndarray, kernel: np.ndarray, **_run_kw) -> np.ndarray:
    x = np.asarray(shareable_energy, np.float32)
    k = np.asarray(kernel, np.float32)
    assert x.shape == (H, W), x.shape
    nc = _build()
    x16 = x.astype(np.float16)
    bands = _make_bands(k)
    in_maps = [_make_core_inputs(x16, bands, core) for core in range(N_CORES)]
    r = run_bass_kernel_spmd(nc, in_maps, core_ids=list(range(N_CORES)), **_run_kw)
    out = np.concatenate(
        [res["out"].astype(np.float32) for res in r.results], axis=0
    )
    if _run_kw:
        _CACHE["last_result"] = r
    return out


# revision 55
# speedup vs baseline: 1.0554x; 1.0554x over previous
"""Trainium2 Bass kernel for nn_EnergyDistributionCNN (3x3 conv -> unfold ->
softmax over patch -> weighted -> fold overlap-add), 8 NeuronCores.

Math (algebraically identical to the torch/jax reference):
    out = conv3x3(x, k)            cross-correlation, zero pad 1
    E   = exp(out)                 (pad pixels contribute exp(0)=1)
    Z   = boxsum3x3(E with ones at pad)
    U   = x / Z
    S   = boxsum3x3(U zero-padded)
    result = E * S

Sharding: row-block across 8 cores with a 3-row halo sliced on the host
(zero-filled at the global edges) -- no device-to-device communication.

Pipeline is fp16 end-to-end: the host casts x to fp16 (rel err 2.4e-4,
well inside the 2e-2 tolerance; measured end-to-end pipeline error is
~3e-3), which halves HBM traffic, runs the PE at full rate and unlocks
the DVE 2x packed-16-bit mode.  Work is spread over all five engines:

  PE   : conv (3 band-matmul passes), Z box (3 shifted ones-band passes,
         or 1 pass on 'mix' tiles), S vertical (1 pass)
  Act  : exp from conv's PSUM (with the row-validity mask folded into the
         scale operand: exp(0*junk)=1), and the S PSUM->fp16 copy
  DVE  : U = x/Z as a single tensor_tensor divide straight from PSUM,
         second horizontal add of the U box, final res = E*S fp16 mul,
         (on 'mix' tiles also the horizontal E adds)
  Pool : first horizontal add of the U box (GpSimd is otherwise idle)
  DMA  : fp16 loads/stores

Row mapping keeps every compute op at partition base 0 (hardware only
allows compute APs to start at partitions 0/32/64/96): the banded
matrices alternate lower/upper diagonals so each stage's output lands
re-centred, and frame-edge partitions hold junk that is either masked
(exp scale), harmless (finite, unused), or skipped by the output DMA
(which may start at any partition).
"""

from contextlib import ExitStack

import numpy as np

import concourse.bacc as bacc
import concourse.mybir as mybir
import concourse.tile as tile
from concourse._compat import with_exitstack
from concourse.bass_utils import run_bass_kernel_spmd
from concourse import dve_ops as _dve_ops
from concourse.dve_spec import AluOp as _AluOp
from concourse.dve_spec import Bin as _Bin
from concourse.dve_spec import C0 as _C0
from concourse.dve_spec import C1 as _C1
from concourse.dve_spec import Spec as _Spec
from concourse.dve_spec import Src0 as _Src0
from concourse.dve_spec import Src1 as _Src1
from concourse.dve_spec import lower as _dve_lower
from concourse.dve_uop import DveOpSpec as _DveOpSpec


def _register_recip1nr_mul():
    """Fused out = in1 * approx(1/in0) as ONE custom-DVE op: the fp32
    exponent-flip seed (x*bitcast(~x) lands in [-4.5,-4]), one Chebyshev
    scale and a single Newton pass (~1.7e-3 rel err, fine for the 2e-2
    tolerance), then the multiply -- 6 ALU stages, fits the 8-stage pipe.
    Replaces reciprocal_approx_fast + tensor_mul (halves the DVE cost of
    the U = x/Z stage)."""
    name = "RECIP1NR_MUL_ANT"
    for op in _dve_ops.OPS:
        if op.name == name:
            return op
    _not = _Bin(_AluOp.BITWISE_NOT, _Src0, _Src0)
    _y0 = _not * _C0
    _y1 = _y0 * (_C1 - _Src0 * _y0)

    def _ref(in0, in1, c0, c1, c2):
        not_x = (~np.asarray(in0, np.float32).view(np.int32)).view(np.float32)
        y0 = not_x * np.float32(c0)
        y1 = y0 * (np.float32(c1) - in0 * y0)
        return y1 * in1

    spec = _Spec(body=_y1 * _Src1, reference=_ref)
    row = max(_dve_ops._SUB_OPCODE_FOR_NAME.values()) + 1
    assert row < 0x20
    _dve_ops._SUB_OPCODE_FOR_NAME[name] = row
    shas = {}
    for ver in ("v3", "v4"):
        try:
            uops = _dve_lower(spec, ver=ver)
        except Exception:
            continue
        shas[ver] = _DveOpSpec(name=name, opcode=row, uops=uops, rd1_en=True).sha(ver)
    op = _dve_ops.DveOp(name, spec, subdim=False, uops_sha=shas)
    _dve_ops.OPS.append(op)
    _dve_ops.CUSTOM_DVE_SPECS[name] = spec
    return op


RECIP1NR_MUL = _register_recip1nr_mul()
RECIP_C = _dve_ops.RECIP_APPROX_FAST_CONSTS

F16 = mybir.dt.float16
F32 = mybir.dt.float32

H = 4096
W = 4096
N_CORES = 8
RC = H // N_CORES  # rows per core
HALO = 3
RT = 122           # output rows per normal row-tile (RT + 6 <= 128)
C = 512            # matmul column chunk = one fp32 PSUM bank
CG = 1024          # conv PSUM group (2 banks) -> fewer Act instructions
SEGW = 1024        # folded-tile width segment (4 segs on 32-row blocks)

# Which normal tiles compute the Z box via horizontal adds + 1 matmul
# instead of 3 shifted matmuls on the PE (engine balancing knob).
ZMIX = (False, True, False, False)

XW = W + 2 * HALO        # X cols:  j   <-> global col j-3   (4102)
EW = W + 4               # E cols:  e   <-> global col e-2   (4100)
ZW = W + 2               # Z/U cols: c  <-> global col c-1   (4098)


# ---------------------------------------------------------------- host side

def _band(vals, lo):
    """128x128 band matrix: b[p, m] = vals[p-m-lo] for p-m-lo in 0..2."""
    b = np.zeros((128, 128), np.float32)
    idx = np.arange(128)
    for d in range(3):
        off = lo + d
        p = idx[off:] if off >= 0 else idx[: 128 + off]
        m = p - off
        b[p, m] = vals[d]
    return b


def _make_bands(k: np.ndarray) -> np.ndarray:
    """bands[0..2]: conv lhsT per column-shift v (b[p,m]=k[p-m, v]);
    bands[3]: BT ones, lhsT[m,p]=1 for p-m in 0..2 (Z: E-frame -> X-frame);
    bands[4]: BS ones, lhsT[p,m]=1 for p-m in 0..2 (S: X-frame -> E-frame);
    bands[5..9]: the same five as 4x 32x32 block-diagonals (folded tile)."""
    bands = np.zeros((10, 128, 128), np.float32)
    for v in range(3):
        bands[v] = _band(k[:, v], 0)
    bands[3] = _band([1.0, 1.0, 1.0], 0).T
    bands[4] = _band([1.0, 1.0, 1.0], 0)
    for i in range(5):
        for b in range(4):
            s = slice(32 * b, 32 * b + 32)
            bands[5 + i][s, s] = bands[i][:32, :32]
    # p-major [128, 10*128] so the upload is one contiguous 2560B/partition DMA
    return np.ascontiguousarray(bands.transpose(1, 0, 2).reshape(128, -1)).astype(
        np.float16
    )


def _make_core_inputs(x16: np.ndarray, bands: np.ndarray, core: int):
    r0 = core * RC
    lo, hi = r0 - HALO, r0 + RC + HALO
    xh = np.zeros((RC + 2 * HALO, XW), np.float16)
    s_lo, s_hi = max(lo, 0), min(hi, H)
    xh[s_lo - lo : s_hi - lo, HALO : HALO + W] = x16[s_lo:s_hi]
    gl = np.arange(lo, hi)
    mask = ((gl >= 0) & (gl < H)).astype(np.float32)[:, None]
    # folded-tile inputs pre-blocked on the host: 32-row x (SEGW+6)-col
    # window per 32-partition block -> single contiguous DMAs on device
    fo = RC - 24
    xf = np.zeros((128, SEGW + 6), np.float16)
    mf = np.zeros((128, 1), np.float32)
    for b in range(4):
        rows = xh[fo : fo + 32, b * SEGW : b * SEGW + SEGW + 6]
        xf[32 * b : 32 * b + rows.shape[0], :] = rows
        mf[32 * b : 32 * b + 28, 0] = mask[fo + 1 : fo + 29, 0]
    return {"xh": xh, "mask": mask, "bands": bands, "xf": xf, "mkf": mf}


def _chunks(total: int, step: int):
    out = []
    s = 0
    while s < total:
        out.append((s, min(step, total - s)))
        s += step
    return out


# -------------------------------------------------------------- device side

@with_exitstack
def _energy_body(ctx: ExitStack, tc, out_d, xh_d, mask_d, bands_d, xf_d, mkf_d):
    nc = tc.nc
    Exp = mybir.ActivationFunctionType.Exp
    Copy = mybir.ActivationFunctionType.Copy

    def u_div(out_ap, z_ap, x_ap):
        # U = x * approx(1/Z), one fused custom-DVE instruction straight
        # from Z's PSUM (fp16 output: the DVE output stage downconverts,
        # the fp32 bit-trick only concerns in0)
        nc.vector._custom_dve(
            RECIP1NR_MUL, out=out_ap, in0=z_ap, in1=x_ap,
            s0=RECIP_C["s0"], s1=RECIP_C["s1"], imm2=0.0,
        )

    consts = ctx.enter_context(tc.tile_pool(name="consts", bufs=1))
    bigb = consts.tile([128, 10 * 128], F16, name="bigb")
    nc.sync.dma_start(out=bigb, in_=bands_d)
    # warm the PE p-state on zeroed scratch while the first DMAs land
    wl = consts.tile([128, 128], F16, name="wl")
    wr = consts.tile([128, C], F16, name="wr")
    nc.vector.memset(wl, 0.0)
    nc.vector.memset(wr, 0.0)
    Mv = [bigb[:, i * 128 : (i + 1) * 128] for i in range(3)]
    BT = bigb[:, 3 * 128 : 4 * 128]
    BS = bigb[:, 4 * 128 : 5 * 128]
    MvF = [bigb[:, (5 + i) * 128 : (6 + i) * 128] for i in range(3)]
    BTF = bigb[:, 8 * 128 : 9 * 128]
    BSF = bigb[:, 9 * 128 : 10 * 128]

    xpool = ctx.enter_context(tc.tile_pool(name="xp", bufs=3))
    epool = ctx.enter_context(tc.tile_pool(name="ep", bufs=1))
    ehpool = ctx.enter_context(tc.tile_pool(name="ehp", bufs=2))
    upool = ctx.enter_context(tc.tile_pool(name="up", bufs=2))
    uhpool = ctx.enter_context(tc.tile_pool(name="uhp", bufs=2))
    spool = ctx.enter_context(tc.tile_pool(name="sp", bufs=2))
    respool = ctx.enter_context(tc.tile_pool(name="resp", bufs=3))
    mpool = ctx.enter_context(tc.tile_pool(name="mp", bufs=2))
    ps_c = ctx.enter_context(tc.tile_pool(name="psc", bufs=2, space="PSUM"))
    ps_z = ctx.enter_context(tc.tile_pool(name="psz", bufs=2, space="PSUM"))
    ps_s = ctx.enter_context(tc.tile_pool(name="pss", bufs=2, space="PSUM"))

    E_tiles = [epool.tile([128, EW], F16, name=f"E{i}") for i in range(3)]

    def normal_early(o, R, ti, zmix):
        """DMA, conv+exp, Z box, reciprocal.  Returns state for the late
        stage."""
        P = R + 4  # working partitions (E frame); X uses R+6
        mk = mpool.tile([128, 1], F32, tag="mk")
        nc.sync.dma_start(out=mk[:P], in_=mask_d[o + 1 : o + 1 + P, :])

        X = xpool.tile([128, XW], F16, tag="X")
        # quarters, so the first conv groups start after 1/4 of the transfer
        xq = [0, 2 + W // 4, 2 + W // 2, 2 + 3 * W // 4, XW]
        for a, b in zip(xq[:-1], xq[1:]):
            nc.sync.dma_start(out=X[: R + 6, a:b], in_=xh_d[o : o + R + 6, a:b])

        # conv + exp -> E[m, e] <-> (row o-2+m, col e-2); only the interior
        # e in [2, EW-2) is computed, the pad columns stay at their
        # pre-initialized exp(0)=1
        E = E_tiles[ti % 3]
        for g0, gl in _chunks(W, CG):
            pc = ps_c.tile([128, CG], F32, tag="pc")
            for cs, cl in _chunks(gl, C):
                for v in range(3):
                    nc.tensor.matmul(
                        pc[:P, cs : cs + cl],
                        Mv[v][: R + 6, :P],
                        X[: R + 6, g0 + 2 + cs + v : g0 + 2 + cs + v + cl],
                        start=(v == 0),
                        stop=(v == 2),
                    )
            nc.scalar.activation(
                E[:P, g0 + 2 : g0 + 2 + gl], pc[:P, :gl], Exp, scale=mk[:P]
            )

        if zmix:
            # horizontal E box on DVE in halves, vertical on PE (1 pass)
            eh1 = ehpool.tile([128, ZW], F16, tag="eh1")
            eh = ehpool.tile([128, ZW], F16, tag="eh")
            for h0, hl in ((0, ZW // 2), (ZW // 2, ZW - ZW // 2)):
                nc.vector.tensor_add(
                    out=eh1[:P, h0 : h0 + hl],
                    in0=E[:P, h0 : h0 + hl],
                    in1=E[:P, h0 + 1 : h0 + 1 + hl],
                )
                nc.vector.tensor_add(
                    out=eh[:P, h0 : h0 + hl],
                    in0=eh1[:P, h0 : h0 + hl],
                    in1=E[:P, h0 + 2 : h0 + 2 + hl],
                )

        # Z (X frame via BT), U = x/Z fused straight from PSUM
        U = upool.tile([128, ZW], F16, tag="U")
        for cs, cl in _chunks(ZW, C):
            pz = ps_z.tile([128, C], F32, tag="pz")
            if zmix:
                nc.tensor.matmul(
                    pz[:P, :cl], BT[:P, :P], eh[:P, cs : cs + cl],
                    start=True, stop=True,
                )
            else:
                for v in range(3):
                    nc.tensor.matmul(
                        pz[:P, :cl], BT[:P, :P], E[:P, cs + v : cs + v + cl],
                        start=(v == 0), stop=(v == 2),
                    )
            u_div(U[:P, cs : cs + cl], pz[:P, :cl], X[:P, cs + 2 : cs + 2 + cl])
        return o, R, P, X, E, U

    def normal_late(state, nq):
        """Horizontal U box (first add on the otherwise-idle GpSimd), S
        vertical (E frame via BS), PSUM -> fp16 via Act copy, res = E*S.
        Split into column quarters so the chain pipelines at ~1us
        granularity and the PE never starves long enough to drop out of
        its warm p-state."""
        o, R, P, X, E, U = state
        uh1 = uhpool.tile([128, W], F16, tag="uh1")
        uh = uhpool.tile([128, W], F16, tag="uh")
        S16 = spool.tile([128, W], F16, tag="S16")
        res = respool.tile([128, W], F16, tag="res")
        QW = W // nq
        for qi in range(nq):
            h0 = qi * QW
            nc.gpsimd.tensor_add(
                out=uh1[:P, h0 : h0 + QW],
                in0=U[:P, h0 : h0 + QW],
                in1=U[:P, h0 + 1 : h0 + 1 + QW],
            )
            nc.vector.tensor_add(
                out=uh[:P, h0 : h0 + QW],
                in0=uh1[:P, h0 : h0 + QW],
                in1=U[:P, h0 + 2 : h0 + 2 + QW],
            )
            for cs, cl in _chunks(QW, C):
                ps = ps_s.tile([128, C], F32, tag="ps")
                nc.tensor.matmul(
                    ps[: R + 2, :cl], BS[:P, : R + 2],
                    uh[:P, h0 + cs : h0 + cs + cl],
                    start=True, stop=True,
                )
                nc.scalar.activation(
                    S16[: R + 2, h0 + cs : h0 + cs + cl], ps[: R + 2, :cl], Copy
                )
            nc.vector.tensor_mul(
                out=res[: R + 2, h0 : h0 + QW],
                in0=E[: R + 2, h0 + 2 : h0 + 2 + QW],
                in1=S16[: R + 2, h0 : h0 + QW],
            )
            if qi % 2 == 1 or nq == 8:
                # valid output rows sit at partitions [2, R+2); one DMA per
                # half keeps the serialized HWDGE issue cost down
                w0 = h0 if nq == 8 else h0 - QW
                # issue from the Act queue: its wait (on resmul, just done)
                # is short there, and long input-load waits on the SP queue
                # can no longer block result stores (res-slot backpressure)
                nc.sync.dma_start(
                    out=out_d[o : o + R, w0 : h0 + QW],
                    in_=res[2 : R + 2, w0 : h0 + QW],
                )

    def fold_unit(o, R, p0):
        """The folded last row-tile: four 1024-wide segments stacked on
        32-partition blocks, block-diagonal bands, inputs pre-blocked on
        the host (single DMAs).  Emitted twice -- at both pipeline edges
        for fast fill/drain -- with each emission storing only the two
        segments at [p0, p0+64).  The compute is cheap enough (~1us per
        engine) that duplicating it beats quadrant-sliced matmuls."""
        mk = mpool.tile([128, 1], F32, tag="mkf")
        nc.sync.dma_start(out=mk, in_=mkf_d)
        X = xpool.tile([128, SEGW + 6], F16, tag="Xf")
        nc.sync.dma_start(out=X, in_=xf_d)

        ew, zw = SEGW + 4, SEGW + 2
        E = epool.tile([128, ew], F16, tag="Ef")
        for g0, gl in _chunks(ew, CG):
            pc = ps_c.tile([128, CG], F32, tag="pc")
            for cs, cl in _chunks(gl, C):
                for v in range(3):
                    nc.tensor.matmul(
                        pc[:, cs : cs + cl],
                        MvF[v],
                        X[:, g0 + cs + v : g0 + cs + v + cl],
                        start=(v == 0),
                        stop=(v == 2),
                    )
            nc.scalar.activation(E[:, g0 : g0 + gl], pc[:, :gl], Exp, scale=mk)
        nc.vector.memset(E[0:32, 1:2], 1.0)
        nc.vector.memset(E[96:128, ew - 2 : ew - 1], 1.0)

        U = upool.tile([128, zw], F16, tag="Uf")
        for cs, cl in _chunks(zw, C):
            pz = ps_z.tile([128, C], F32, tag="pz")
            for v in range(3):
                nc.tensor.matmul(
                    pz[:, :cl], BTF, E[:, cs + v : cs + v + cl],
                    start=(v == 0), stop=(v == 2),
                )
            u_div(U[:, cs : cs + cl], pz[:, :cl], X[:, cs + 2 : cs + 2 + cl])
        uh1 = uhpool.tile([128, SEGW], F16, tag="uh1f")
        nc.gpsimd.tensor_add(out=uh1, in0=U[:, 0:SEGW], in1=U[:, 1 : SEGW + 1])
        uh = uhpool.tile([128, SEGW], F16, tag="uhf")
        nc.vector.tensor_add(out=uh, in0=uh1, in1=U[:, 2 : SEGW + 2])

        S16 = spool.tile([128, SEGW], F16, tag="S16f")
        for cs, cl in _chunks(SEGW, C):
            ps = ps_s.tile([128, C], F32, tag="ps")
            nc.tensor.matmul(ps[:, :cl], BSF, uh[:, cs : cs + cl], start=True, stop=True)
            nc.scalar.activation(S16[:, cs : cs + cl], ps[:, :cl], Copy)
        res = respool.tile([128, SEGW], F16, tag="resf")
        nc.vector.tensor_mul(out=res, in0=E[:, 2 : SEGW + 2], in1=S16)
        return res

    def fold_store(res, o, R, blocks):
        for b in blocks:
            nc.sync.dma_start(
                out=out_d[o : o + R, b * SEGW : (b + 1) * SEGW],
                in_=res[32 * b + 2 : 32 * b + 2 + R, :],
            )

    with nc.allow_low_precision("fp16 pipeline; verified within tolerance"):
        # pre-initialized E tiles, rotated manually: the pad columns
        # (exp(0)=1 outside the grid) are set once and exp only ever writes
        # the interior, so no per-tile DVE memset sits in front of the next
        # tile's Z matmuls
        for Et in E_tiles:
            nc.vector.memset(Et[:, 0:2], 1.0)
            nc.vector.memset(Et[:, EW - 2 : EW], 1.0)
        for i in range(6):
            pw = ps_z.tile([128, C], F32, tag="pz")
            nc.tensor.matmul(pw, wl, wr, start=True, stop=True)

        tiles = _chunks(RC, RT)
        fo, fr = tiles[-1]
        assert fr <= 26
        # folded tile first (cheap fill); half its stores drain at the end
        resf = fold_unit(fo, fr, 0)
        fold_store(resf, fo, fr, (0, 1))
        # 2-stage software pipeline: emit tile t+1's early stage before
        # tile t's late stage so every in-order engine queue (Act
        # especially: exp(t+1) must not sit behind Scopy(t)) has next-tile
        # work ahead of the current tile's chain tail
        nbig = len(tiles) - 1
        pending = None
        for i, (o, R) in enumerate(tiles[:-1]):
            st = normal_early(o, R, i, ZMIX[i % len(ZMIX)])
            if pending is not None:
                normal_late(*pending)
            pending = (st, 8 if i == nbig - 1 else 4)
        normal_late(*pending)
        fold_store(resf, fo, fr, (2, 3))


_CACHE: dict = {}


def _build():
    if "nc" in _CACHE:
        return _CACHE["nc"]
    nc = bacc.Bacc(
        "TRN2", target_bir_lowering=False, debug=False, num_devices=N_CORES
    )
    xh_d = nc.dram_tensor("xh", (RC + 2 * HALO, XW), F16, kind="ExternalInput").ap()
    mask_d = nc.dram_tensor("mask", (RC + 2 * HALO, 1), F32, kind="ExternalInput").ap()
    bands_d = nc.dram_tensor("bands", (128, 10 * 128), F16, kind="ExternalInput").ap()
    xf_d = nc.dram_tensor("xf", (128, SEGW + 6), F16, kind="ExternalInput").ap()
    mkf_d = nc.dram_tensor("mkf", (128, 1), F32, kind="ExternalInput").ap()
    out_d = nc.dram_tensor("out", (RC, W), F16, kind="ExternalOutput").ap()
    with tile.TileContext(nc) as tc:
        _energy_body(tc, out_d, xh_d, mask_d, bands_d, xf_d, mkf_d)
    nc.compile()
    _CACHE["nc"] = nc
    return nc


def kernel(shareable_energy: np.ndarray, kernel: np.ndarray, **_run_kw) -> np.ndarray:
    x = np.asarray(shareable_energy, np.float32)
    k = np.asarray(kernel, np.float32)
    assert x.shape == (H, W), x.shape
    nc = _build()
    x16 = x.astype(np.float16)
    bands = _make_bands(k)
    in_maps = [_make_core_inputs(x16, bands, core) for core in range(N_CORES)]
    r = run_bass_kernel_spmd(nc, in_maps, core_ids=list(range(N_CORES)), **_run_kw)
    out = np.concatenate(
        [res["out"].astype(np.float32) for res in r.results], axis=0
    )
    if _run_kw:
        _CACHE["last_result"] = r
    return out


# revision 56
# speedup vs baseline: 1.0556x; 1.0003x over previous
"""Trainium2 Bass kernel for nn_EnergyDistributionCNN (3x3 conv -> unfold ->
softmax over patch -> weighted -> fold overlap-add), 8 NeuronCores.

Math (algebraically identical to the torch/jax reference):
    out = conv3x3(x, k)            cross-correlation, zero pad 1
    E   = exp(out)                 (pad pixels contribute exp(0)=1)
    Z   = boxsum3x3(E with ones at pad)
    U   = x / Z
    S   = boxsum3x3(U zero-padded)
    result = E * S

Sharding: row-block across 8 cores with a 3-row halo sliced on the host
(zero-filled at the global edges) -- no device-to-device communication.

Pipeline is fp16 end-to-end: the host casts x to fp16 (rel err 2.4e-4,
well inside the 2e-2 tolerance; measured end-to-end pipeline error is
~3e-3), which halves HBM traffic, runs the PE at full rate and unlocks
the DVE 2x packed-16-bit mode.  Work is spread over all five engines:

  PE   : conv (3 band-matmul passes), Z box (3 shifted ones-band passes,
         or 1 pass on 'mix' tiles), S vertical (1 pass)
  Act  : exp from conv's PSUM (with the row-validity mask folded into the
         scale operand: exp(0*junk)=1), and the S PSUM->fp16 copy
  DVE  : U = x/Z as a single tensor_tensor divide straight from PSUM,
         second horizontal add of the U box, final res = E*S fp16 mul,
         (on 'mix' tiles also the horizontal E adds)
  Pool : first horizontal add of the U box (GpSimd is otherwise idle)
  DMA  : fp16 loads/stores

Row mapping keeps every compute op at partition base 0 (hardware only
allows compute APs to start at partitions 0/32/64/96): the banded
matrices alternate lower/upper diagonals so each stage's output lands
re-centred, and frame-edge partitions hold junk that is either masked
(exp scale), harmless (finite, unused), or skipped by the output DMA
(which may start at any partition).
"""

from contextlib import ExitStack

import numpy as np

import concourse.bacc as bacc
import concourse.mybir as mybir
import concourse.tile as tile
from concourse._compat import with_exitstack
from concourse.bass_utils import run_bass_kernel_spmd
from concourse import dve_ops as _dve_ops
from concourse.dve_spec import AluOp as _AluOp
from concourse.dve_spec import Bin as _Bin
from concourse.dve_spec import C0 as _C0
from concourse.dve_spec import C1 as _C1
from concourse.dve_spec import Spec as _Spec
from concourse.dve_spec import Src0 as _Src0
from concourse.dve_spec import Src1 as _Src1
from concourse.dve_spec import lower as _dve_lower
from concourse.dve_uop import DveOpSpec as _DveOpSpec


def _register_recip1nr_mul():
    """Fused out = in1 * approx(1/in0) as ONE custom-DVE op: the fp32
    exponent-flip seed (x*bitcast(~x) lands in [-4.5,-4]), one Chebyshev
    scale and a single Newton pass (~1.7e-3 rel err, fine for the 2e-2
    tolerance), then the multiply -- 6 ALU stages, fits the 8-stage pipe.
    Replaces reciprocal_approx_fast + tensor_mul (halves the DVE cost of
    the U = x/Z stage)."""
    name = "RECIP1NR_MUL_ANT"
    for op in _dve_ops.OPS:
        if op.name == name:
            return op
    _not = _Bin(_AluOp.BITWISE_NOT, _Src0, _Src0)
    _y0 = _not * _C0
    _y1 = _y0 * (_C1 - _Src0 * _y0)

    def _ref(in0, in1, c0, c1, c2):
        not_x = (~np.asarray(in0, np.float32).view(np.int32)).view(np.float32)
        y0 = not_x * np.float32(c0)
        y1 = y0 * (np.float32(c1) - in0 * y0)
        return y1 * in1

    spec = _Spec(body=_y1 * _Src1, reference=_ref)
    row = max(_dve_ops._SUB_OPCODE_FOR_NAME.values()) + 1
    assert row < 0x20
    _dve_ops._SUB_OPCODE_FOR_NAME[name] = row
    shas = {}
    for ver in ("v3", "v4"):
        try:
            uops = _dve_lower(spec, ver=ver)
        except Exception:
            continue
        shas[ver] = _DveOpSpec(name=name, opcode=row, uops=uops, rd1_en=True).sha(ver)
    op = _dve_ops.DveOp(name, spec, subdim=False, uops_sha=shas)
    _dve_ops.OPS.append(op)
    _dve_ops.CUSTOM_DVE_SPECS[name] = spec
    return op


RECIP1NR_MUL = _register_recip1nr_mul()
RECIP_C = _dve_ops.RECIP_APPROX_FAST_CONSTS

F16 = mybir.dt.float16
F32 = mybir.dt.float32

H = 4096
W = 4096
N_CORES = 8
RC = H // N_CORES  # rows per core
HALO = 3
RT = 122           # output rows per normal row-tile (RT + 6 <= 128)
C = 512            # matmul column chunk = one fp32 PSUM bank
CG = 1024          # conv PSUM group (2 banks) -> fewer Act instructions
SEGW = 1024        # folded-tile width segment (4 segs on 32-row blocks)

# Which normal tiles compute the Z box via horizontal adds + 1 matmul
# instead of 3 shifted matmuls on the PE (engine balancing knob).
ZMIX = (False, True, False, False)

XW = W + 2 * HALO        # X cols:  j   <-> global col j-3   (4102)
EW = W + 4               # E cols:  e   <-> global col e-2   (4100)
ZW = W + 2               # Z/U cols: c  <-> global col c-1   (4098)


# ---------------------------------------------------------------- host side

def _band(vals, lo):
    """128x128 band matrix: b[p, m] = vals[p-m-lo] for p-m-lo in 0..2."""
    b = np.zeros((128, 128), np.float32)
    idx = np.arange(128)
    for d in range(3):
        off = lo + d
        p = idx[off:] if off >= 0 else idx[: 128 + off]
        m = p - off
        b[p, m] = vals[d]
    return b


def _make_bands(k: np.ndarray) -> np.ndarray:
    """bands[0..2]: conv lhsT per column-shift v (b[p,m]=k[p-m, v]);
    bands[3]: BT ones, lhsT[m,p]=1 for p-m in 0..2 (Z: E-frame -> X-frame);
    bands[4]: BS ones, lhsT[p,m]=1 for p-m in 0..2 (S: X-frame -> E-frame);
    bands[5..9]: the same five as 4x 32x32 block-diagonals (folded tile)."""
    bands = np.zeros((10, 128, 128), np.float32)
    for v in range(3):
        bands[v] = _band(k[:, v], 0)
    bands[3] = _band([1.0, 1.0, 1.0], 0).T
    bands[4] = _band([1.0, 1.0, 1.0], 0)
    for i in range(5):
        for b in range(4):
            s = slice(32 * b, 32 * b + 32)
            bands[5 + i][s, s] = bands[i][:32, :32]
    # p-major [128, 10*128] so the upload is one contiguous 2560B/partition DMA
    return np.ascontiguousarray(bands.transpose(1, 0, 2).reshape(128, -1)).astype(
        np.float16
    )


def _make_core_inputs(x16: np.ndarray, bands: np.ndarray, core: int):
    r0 = core * RC
    lo, hi = r0 - HALO, r0 + RC + HALO
    xh = np.zeros((RC + 2 * HALO, XW), np.float16)
    s_lo, s_hi = max(lo, 0), min(hi, H)
    xh[s_lo - lo : s_hi - lo, HALO : HALO + W] = x16[s_lo:s_hi]
    gl = np.arange(lo, hi)
    mask = ((gl >= 0) & (gl < H)).astype(np.float32)[:, None]
    # folded-tile inputs pre-blocked on the host: 32-row x (SEGW+6)-col
    # window per 32-partition block -> single contiguous DMAs on device
    fo = RC - 24
    xf = np.zeros((128, SEGW + 6), np.float16)
    mf = np.zeros((128, 1), np.float32)
    for b in range(4):
        rows = xh[fo : fo + 32, b * SEGW : b * SEGW + SEGW + 6]
        xf[32 * b : 32 * b + rows.shape[0], :] = rows
        mf[32 * b : 32 * b + 28, 0] = mask[fo + 1 : fo + 29, 0]
    return {"xh": xh, "mask": mask, "bands": bands, "xf": xf, "mkf": mf}


def _chunks(total: int, step: int):
    out = []
    s = 0
    while s < total:
        out.append((s, min(step, total - s)))
        s += step
    return out


# -------------------------------------------------------------- device side

@with_exitstack
def _energy_body(ctx: ExitStack, tc, out_d, xh_d, mask_d, bands_d, xf_d, mkf_d):
    nc = tc.nc
    Exp = mybir.ActivationFunctionType.Exp
    Copy = mybir.ActivationFunctionType.Copy

    def u_div(out_ap, z_ap, x_ap):
        # U = x * approx(1/Z), one fused custom-DVE instruction straight
        # from Z's PSUM (fp16 output: the DVE output stage downconverts,
        # the fp32 bit-trick only concerns in0)
        nc.vector._custom_dve(
            RECIP1NR_MUL, out=out_ap, in0=z_ap, in1=x_ap,
            s0=RECIP_C["s0"], s1=RECIP_C["s1"], imm2=0.0,
        )

    consts = ctx.enter_context(tc.tile_pool(name="consts", bufs=1))
    bigb = consts.tile([128, 10 * 128], F16, name="bigb")
    nc.sync.dma_start(out=bigb, in_=bands_d)
    # warm the PE p-state on zeroed scratch while the first DMAs land
    wl = consts.tile([128, 128], F16, name="wl")
    wr = consts.tile([128, C], F16, name="wr")
    nc.vector.memset(wl, 0.0)
    nc.vector.memset(wr, 0.0)
    Mv = [bigb[:, i * 128 : (i + 1) * 128] for i in range(3)]
    BT = bigb[:, 3 * 128 : 4 * 128]
    BS = bigb[:, 4 * 128 : 5 * 128]
    MvF = [bigb[:, (5 + i) * 128 : (6 + i) * 128] for i in range(3)]
    BTF = bigb[:, 8 * 128 : 9 * 128]
    BSF = bigb[:, 9 * 128 : 10 * 128]

    xpool = ctx.enter_context(tc.tile_pool(name="xp", bufs=3))
    epool = ctx.enter_context(tc.tile_pool(name="ep", bufs=1))
    ehpool = ctx.enter_context(tc.tile_pool(name="ehp", bufs=2))
    upool = ctx.enter_context(tc.tile_pool(name="up", bufs=2))
    uhpool = ctx.enter_context(tc.tile_pool(name="uhp", bufs=2))
    spool = ctx.enter_context(tc.tile_pool(name="sp", bufs=2))
    respool = ctx.enter_context(tc.tile_pool(name="resp", bufs=3))
    mpool = ctx.enter_context(tc.tile_pool(name="mp", bufs=2))
    ps_c = ctx.enter_context(tc.tile_pool(name="psc", bufs=2, space="PSUM"))
    ps_z = ctx.enter_context(tc.tile_pool(name="psz", bufs=2, space="PSUM"))
    ps_s = ctx.enter_context(tc.tile_pool(name="pss", bufs=2, space="PSUM"))

    E_tiles = [epool.tile([128, EW], F16, name=f"E{i}") for i in range(3)]

    def normal_early(o, R, ti, zmix):
        """DMA, conv+exp, Z box, reciprocal.  Returns state for the late
        stage."""
        P = R + 4  # working partitions (E frame); X uses R+6
        mk = mpool.tile([128, 1], F32, tag="mk")
        nc.sync.dma_start(out=mk[:P], in_=mask_d[o + 1 : o + 1 + P, :])

        X = xpool.tile([128, XW], F16, tag="X")
        # quarters, so the first conv groups start after 1/4 of the transfer
        xq = [0, 2 + W // 4, 2 + W // 2, 2 + 3 * W // 4, XW]
        for a, b in zip(xq[:-1], xq[1:]):
            nc.sync.dma_start(out=X[: R + 6, a:b], in_=xh_d[o : o + R + 6, a:b])

        # conv + exp -> E[m, e] <-> (row o-2+m, col e-2); only the interior
        # e in [2, EW-2) is computed, the pad columns stay at their
        # pre-initialized exp(0)=1
        E = E_tiles[ti % 3]
        for g0, gl in _chunks(W, CG):
            pc = ps_c.tile([128, CG], F32, tag="pc")
            for cs, cl in _chunks(gl, C):
                for v in range(3):
                    nc.tensor.matmul(
                        pc[:P, cs : cs + cl],
                        Mv[v][: R + 6, :P],
                        X[: R + 6, g0 + 2 + cs + v : g0 + 2 + cs + v + cl],
                        start=(v == 0),
                        stop=(v == 2),
                    )
            nc.scalar.activation(
                E[:P, g0 + 2 : g0 + 2 + gl], pc[:P, :gl], Exp, scale=mk[:P]
            )

        if zmix:
            # horizontal E box on DVE in halves, vertical on PE (1 pass)
            eh1 = ehpool.tile([128, ZW], F16, tag="eh1")
            eh = ehpool.tile([128, ZW], F16, tag="eh")
            for h0, hl in ((0, ZW // 2), (ZW // 2, ZW - ZW // 2)):
                nc.vector.tensor_add(
                    out=eh1[:P, h0 : h0 + hl],
                    in0=E[:P, h0 : h0 + hl],
                    in1=E[:P, h0 + 1 : h0 + 1 + hl],
                )
                nc.vector.tensor_add(
                    out=eh[:P, h0 : h0 + hl],
                    in0=eh1[:P, h0 : h0 + hl],
                    in1=E[:P, h0 + 2 : h0 + 2 + hl],
                )

        # Z (X frame via BT), U = x/Z fused straight from PSUM
        U = upool.tile([128, ZW], F16, tag="U")
        for cs, cl in _chunks(ZW, C):
            pz = ps_z.tile([128, C], F32, tag="pz")
            if zmix:
                nc.tensor.matmul(
                    pz[:P, :cl], BT[:P, :P], eh[:P, cs : cs + cl],
                    start=True, stop=True,
                )
            else:
                for v in range(3):
                    nc.tensor.matmul(
                        pz[:P, :cl], BT[:P, :P], E[:P, cs + v : cs + v + cl],
                        start=(v == 0), stop=(v == 2),
                    )
            u_div(U[:P, cs : cs + cl], pz[:P, :cl], X[:P, cs + 2 : cs + 2 + cl])
        return o, R, P, X, E, U

    def normal_late(state, nq):
        """Horizontal U box (first add on the otherwise-idle GpSimd), S
        vertical (E frame via BS), PSUM -> fp16 via Act copy, res = E*S.
        Split into column quarters so the chain pipelines at ~1us
        granularity and the PE never starves long enough to drop out of
        its warm p-state."""
        o, R, P, X, E, U = state
        uh1 = uhpool.tile([128, W], F16, tag="uh1")
        uh = uhpool.tile([128, W], F16, tag="uh")
        S16 = spool.tile([128, W], F16, tag="S16")
        res = respool.tile([128, W], F16, tag="res")
        QW = W // nq
        for qi in range(nq):
            h0 = qi * QW
            eng1 = nc.vector if (nq == 8 and qi % 2 == 1) else nc.gpsimd
            eng1.tensor_add(
                out=uh1[:P, h0 : h0 + QW],
                in0=U[:P, h0 : h0 + QW],
                in1=U[:P, h0 + 1 : h0 + 1 + QW],
            )
            nc.vector.tensor_add(
                out=uh[:P, h0 : h0 + QW],
                in0=uh1[:P, h0 : h0 + QW],
                in1=U[:P, h0 + 2 : h0 + 2 + QW],
            )
            for cs, cl in _chunks(QW, C):
                ps = ps_s.tile([128, C], F32, tag="ps")
                nc.tensor.matmul(
                    ps[: R + 2, :cl], BS[:P, : R + 2],
                    uh[:P, h0 + cs : h0 + cs + cl],
                    start=True, stop=True,
                )
                nc.scalar.activation(
                    S16[: R + 2, h0 + cs : h0 + cs + cl], ps[: R + 2, :cl], Copy
                )
            nc.vector.tensor_mul(
                out=res[: R + 2, h0 : h0 + QW],
                in0=E[: R + 2, h0 + 2 : h0 + 2 + QW],
                in1=S16[: R + 2, h0 : h0 + QW],
            )
            if qi % 2 == 1 or nq == 8:
                # valid output rows sit at partitions [2, R+2); one DMA per
                # half keeps the serialized HWDGE issue cost down
                w0 = h0 if nq == 8 else h0 - QW
                # issue from the Act queue: its wait (on resmul, just done)
                # is short there, and long input-load waits on the SP queue
                # can no longer block result stores (res-slot backpressure)
                nc.sync.dma_start(
                    out=out_d[o : o + R, w0 : h0 + QW],
                    in_=res[2 : R + 2, w0 : h0 + QW],
                )

    def fold_unit(o, R, p0):
        """The folded last row-tile: four 1024-wide segments stacked on
        32-partition blocks, block-diagonal bands, inputs pre-blocked on
        the host (single DMAs).  Emitted twice -- at both pipeline edges
        for fast fill/drain -- with each emission storing only the two
        segments at [p0, p0+64).  The compute is cheap enough (~1us per
        engine) that duplicating it beats quadrant-sliced matmuls."""
        mk = mpool.tile([128, 1], F32, tag="mkf")
        nc.sync.dma_start(out=mk, in_=mkf_d)
        X = xpool.tile([128, SEGW + 6], F16, tag="Xf")
        nc.sync.dma_start(out=X, in_=xf_d)

        ew, zw = SEGW + 4, SEGW + 2
        E = epool.tile([128, ew], F16, tag="Ef")
        for g0, gl in _chunks(ew, CG):
            pc = ps_c.tile([128, CG], F32, tag="pc")
            for cs, cl in _chunks(gl, C):
                for v in range(3):
                    nc.tensor.matmul(
                        pc[:, cs : cs + cl],
                        MvF[v],
                        X[:, g0 + cs + v : g0 + cs + v + cl],
                        start=(v == 0),
                        stop=(v == 2),
                    )
            nc.scalar.activation(E[:, g0 : g0 + gl], pc[:, :gl], Exp, scale=mk)
        nc.vector.memset(E[0:32, 1:2], 1.0)
        nc.vector.memset(E[96:128, ew - 2 : ew - 1], 1.0)

        U = upool.tile([128, zw], F16, tag="Uf")
        for cs, cl in _chunks(zw, C):
            pz = ps_z.tile([128, C], F32, tag="pz")
            for v in range(3):
                nc.tensor.matmul(
                    pz[:, :cl], BTF, E[:, cs + v : cs + v + cl],
                    start=(v == 0), stop=(v == 2),
                )
            u_div(U[:, cs : cs + cl], pz[:, :cl], X[:, cs + 2 : cs + 2 + cl])
        uh1 = uhpool.tile([128, SEGW], F16, tag="uh1f")
        nc.gpsimd.tensor_add(out=uh1, in0=U[:, 0:SEGW], in1=U[:, 1 : SEGW + 1])
        uh = uhpool.tile([128, SEGW], F16, tag="uhf")
        nc.vector.tensor_add(out=uh, in0=uh1, in1=U[:, 2 : SEGW + 2])

        S16 = spool.tile([128, SEGW], F16, tag="S16f")
        for cs, cl in _chunks(SEGW, C):
            ps = ps_s.tile([128, C], F32, tag="ps")
            nc.tensor.matmul(ps[:, :cl], BSF, uh[:, cs : cs + cl], start=True, stop=True)
            nc.scalar.activation(S16[:, cs : cs + cl], ps[:, :cl], Copy)
        res = respool.tile([128, SEGW], F16, tag="resf")
        nc.vector.tensor_mul(out=res, in0=E[:, 2 : SEGW + 2], in1=S16)
        return res

    def fold_store(res, o, R, blocks):
        for b in blocks:
            nc.sync.dma_start(
                out=out_d[o : o + R, b * SEGW : (b + 1) * SEGW],
                in_=res[32 * b + 2 : 32 * b + 2 + R, :],
            )

    with nc.allow_low_precision("fp16 pipeline; verified within tolerance"):
        # pre-initialized E tiles, rotated manually: the pad columns
        # (exp(0)=1 outside the grid) are set once and exp only ever writes
        # the interior, so no per-tile DVE memset sits in front of the next
        # tile's Z matmuls
        for Et in E_tiles:
            nc.vector.memset(Et[:, 0:2], 1.0)
            nc.vector.memset(Et[:, EW - 2 : EW], 1.0)
        for i in range(6):
            pw = ps_z.tile([128, C], F32, tag="pz")
            nc.tensor.matmul(pw, wl, wr, start=True, stop=True)

        tiles = _chunks(RC, RT)
        fo, fr = tiles[-1]
        assert fr <= 26
        # folded tile first (cheap fill); half its stores drain at the end
        resf = fold_unit(fo, fr, 0)
        fold_store(resf, fo, fr, (0, 1))
        # 2-stage software pipeline: emit tile t+1's early stage before
        # tile t's late stage so every in-order engine queue (Act
        # especially: exp(t+1) must not sit behind Scopy(t)) has next-tile
        # work ahead of the current tile's chain tail
        nbig = len(tiles) - 1
        pend = []
        for i, (o, R) in enumerate(tiles[:-1]):
            st = normal_early(o, R, i, ZMIX[i % len(ZMIX)])
            pend.append((st, 8 if i == nbig - 1 else 4))
            if len(pend) > 2:
                normal_late(*pend.pop(0))
        for p in pend:
            normal_late(*p)
        fold_store(resf, fo, fr, (2, 3))


_CACHE: dict = {}


def _build():
    if "nc" in _CACHE:
        return _CACHE["nc"]
    nc = bacc.Bacc(
        "TRN2", target_bir_lowering=False, debug=False, num_devices=N_CORES
    )
    xh_d = nc.dram_tensor("xh", (RC + 2 * HALO, XW), F16, kind="ExternalInput").ap()
    mask_d = nc.dram_tensor("mask", (RC + 2 * HALO, 1), F32, kind="ExternalInput").ap()
    bands_d = nc.dram_tensor("bands", (128, 10 * 128), F16, kind="ExternalInput").ap()
    xf_d = nc.dram_tensor("xf", (128, SEGW + 6), F16, kind="ExternalInput").ap()
    mkf_d = nc.dram_tensor("mkf", (128, 1), F32, kind="ExternalInput").ap()
    out_d = nc.dram_tensor("out", (RC, W), F16, kind="ExternalOutput").ap()
    with tile.TileContext(nc) as tc:
        _energy_body(tc, out_d, xh_d, mask_d, bands_d, xf_d, mkf_d)
    nc.compile()
    _CACHE["nc"] = nc
    return nc


def kernel(shareable_energy: np.ndarray, kernel: np.ndarray, **_run_kw) -> np.ndarray:
    x = np.asarray(shareable_energy, np.float32)
    k = np.asarray(kernel, np.float32)
    assert x.shape == (H, W), x.shape
    nc = _build()
    x16 = x.astype(np.float16)
    bands = _make_bands(k)
    in_maps = [_make_core_inputs(x16, bands, core) for core in range(N_CORES)]
    r = run_bass_kernel_spmd(nc, in_maps, core_ids=list(range(N_CORES)), **_run_kw)
    out = np.concatenate(
        [res["out"].astype(np.float32) for res in r.results], axis=0
    )
    if _run_kw:
        _CACHE["last_result"] = r
    return out
